# revision 37
# baseline (speedup 1.0000x reference)
"""Trainium2 Bass kernel: MultiHeadedAttention block (QKV proj + softmax
attention + output proj + residual + LayerNorm), returning (y, attn).

Sharding: 8 cores = 2 batches x 4 query-chunks of 512. Each core computes
Q-projection for its 512 queries, K/V projections for the full 2048-key
sequence of its batch, all 16 heads of attention for its queries, the
output projection, residual add and LayerNorm. No cross-core communication;
the host slices inputs and concatenates outputs.

Layouts on chip are feature-major ("transposed"): activations live as
[feature_partitions, tokens] so projection matmuls use the weight matrices
in their natural [d_in, d_out] layout as the stationary operand. Scores are
computed twice - once as [q, k] (softmax reductions + attn output) and once
as [k, q] (feeds attn @ V) - which is cheaper than transposing the 16.8M
element probability matrix on-chip.
"""

import numpy as np
import ml_dtypes

import concourse.bass as bass
import concourse.tile as tile
from concourse import mybir
from concourse.bass_utils import run_bass_kernel_spmd
def _legalize_single_wait(nc, maxw=1):
    """This container's walrus rejects instructions carrying more than one
    sync-wait command (setupSyncWait: 'Too many sync wait commands'). Hoist
    extra waits onto same-engine NoOps inserted immediately before the
    instruction: engines dispatch in order, so the ordering is identical."""
    nnop = 0
    for f in nc.m.functions:
        for bb in f.blocks:
            out = []
            changed = False
            for ins in bb.instructions:
                si = ins.sync_info
                waits = list(si.on_wait) if (si is not None and si.on_wait) else []
                if len(waits) > maxw:
                    for w in waits[:-maxw]:
                        nop = mybir.InstNoOp(
                            name=f"waitnop_{nnop}", ins=[], outs=[]
                        )
                        nnop += 1
                        nop.engine = ins.engine
                        nop.sync_info = mybir.SyncInfo(on_wait=[w], on_update=[])
                        nc.register_instruction(nop)
                        out.append(nop)
                    ins.sync_info = mybir.SyncInfo(
                        on_wait=waits[-maxw:], on_update=list(si.on_update or [])
                    )
                    changed = True
                out.append(ins)
            if changed:
                bb.instructions = out

BF16 = ml_dtypes.bfloat16
AF = mybir.ActivationFunctionType
ALU = mybir.AluOpType
F32 = mybir.dt.float32
BF = mybir.dt.bfloat16

N_HEAD = 16
D_K = 64
D_MODEL = 1024
B = 2
SEQ = 2048
N_CORES = 8
Q_SHARD = B * SEQ // N_CORES  # 512
LN_EPS = 1e-6


def _chunks(total, step):
    return [(o, min(step, total - o)) for o in range(0, total, step)]


def build_program(dm=D_MODEL, h=N_HEAD, dk=D_K, sq=Q_SHARD, sk=SEQ, eps=LN_EPS,
                  use_transpose=True, gpsimd_norm=True):
    """Build the per-core Bass program (identical on all cores)."""
    nc = bass.Bass()
    hd = h * dk
    DMT = dm // 128   # d_model tiles (contraction tiles for projections)
    HDT = hd // 128   # head-dim-stack tiles
    QT = sq // 128    # query tiles
    KT = sk // 128    # key tiles
    HPT = 128 // dk   # heads per hd tile (2)
    smax_scale = 1.0 / float(dk) ** 0.5

    # ---- DRAM I/O ----
    qT_d = nc.dram_tensor("qT", [dm, sq], BF, kind="ExternalInput")
    kT_d = nc.dram_tensor("kT", [dm, sk], BF, kind="ExternalInput")
    vT_d = nc.dram_tensor("vT", [dm, sk], BF, kind="ExternalInput")
    res_d = nc.dram_tensor("resid", [sq, dm], F32, kind="ExternalInput")
    Wq_d = nc.dram_tensor("Wq", [dm, hd], BF, kind="ExternalInput")
    Wk_d = nc.dram_tensor("Wk", [dm, hd], BF, kind="ExternalInput")
    Wv_d = nc.dram_tensor("Wv", [dm, hd], BF, kind="ExternalInput")
    Wo_d = nc.dram_tensor("Wo", [hd, dm], BF, kind="ExternalInput")
    bq_d = nc.dram_tensor("bq", [hd], F32, kind="ExternalInput")
    bk_d = nc.dram_tensor("bk", [hd], F32, kind="ExternalInput")
    bv_d = nc.dram_tensor("bv", [hd], F32, kind="ExternalInput")
    bo_d = nc.dram_tensor("bo", [dm], F32, kind="ExternalInput")
    lns_d = nc.dram_tensor("ln_scale", [dm], F32, kind="ExternalInput")
    lnb_d = nc.dram_tensor("ln_bias", [dm], F32, kind="ExternalInput")
    y_d = nc.dram_tensor("y", [sq, dm], F32, kind="ExternalOutput")
    attn_d = nc.dram_tensor("attn", [h, sq, sk], F32, kind="ExternalOutput")

    def bcast_ap(src_ap, parts):
        return bass.AP(
            tensor=src_ap.tensor,
            offset=src_ap.offset,
            ap=[[0, parts]] + [list(d) for d in src_ap.ap],
        )

    with tile.TileContext(nc) as tc:
        with (
            tc.tile_pool(name="psA", bufs=2, space="PSUM") as psA,
            tc.tile_pool(name="psT", bufs=2, space="PSUM") as psT,
            tc.tile_pool(name="psV", bufs=2, space="PSUM") as psV,
            tc.tile_pool(name="consts", bufs=1) as cons,
            tc.tile_pool(name="proj", bufs=1) as projp,
        ):
            # ---- constants ----
            bq_sb = cons.tile([128, HDT], F32, name="bq_sb", tag="bq_sb")
            nc.sync.dma_start(out=bq_sb, in_=bq_d.rearrange("(t p) -> p t", p=128))
            bk_sb = cons.tile([128, HDT], F32, name="bk_sb", tag="bk_sb")
            nc.sync.dma_start(out=bk_sb, in_=bk_d.rearrange("(t p) -> p t", p=128))
            bv_bc = cons.tile([128, hd], F32, name="bv_bc", tag="bv_bc")
            nc.gpsimd.dma_start(out=bv_bc, in_=bcast_ap(bv_d[:], 128))
            bo_bc = cons.tile([128, dm], F32, name="bo_bc", tag="bo_bc")
            nc.gpsimd.dma_start(out=bo_bc, in_=bcast_ap(bo_d[:], 128))
            lns_bc = cons.tile([128, dm], F32, name="lns_bc", tag="lns_bc")
            nc.gpsimd.dma_start(out=lns_bc, in_=bcast_ap(lns_d[:], 128))
            lnb_bc = cons.tile([128, dm], F32, name="lnb_bc", tag="lnb_bc")
            nc.gpsimd.dma_start(out=lnb_bc, in_=bcast_ap(lnb_d[:], 128))
            eps_sb = cons.tile([128, 1], F32, name="eps_sb", tag="eps_sb")
            nc.vector.memset(eps_sb, eps)
            ident = cons.tile([128, 128], BF, name="ident", tag="ident")
            from concourse.masks import make_identity
            make_identity(nc, ident)

            # ---- persistent projection outputs (feature-major) ----
            qhT = projp.tile([128, HDT, sq], BF, name="qhT", tag="qhT")
            khT = projp.tile([128, HDT, sk], BF, name="khT", tag="khT")
            vh = projp.tile([128, KT, hd], BF, name="vh", tag="vh")
            outT = projp.tile([128, HDT, sq], BF, name="outT", tag="outT")

            # ================= Phase 1: QKV projections =================
            with tc.tile_pool(name="ld", bufs=1) as ld:
                wq_sb = ld.tile([128, DMT, hd], BF, name="wq_sb", tag="w", bufs=2)
                wq_r = Wq_d.rearrange("(t p) n -> p t n", p=128)
                qT_sb = ld.tile([128, DMT, sq], BF, name="qT_sb", tag="xq")
                qT_r = qT_d.rearrange("(t p) n -> p t n", p=128)
                for kt in range(DMT):
                    nc.sync.dma_start(out=wq_sb[:, kt, :], in_=wq_r[:, kt, :])
                    nc.sync.dma_start(out=qT_sb[:, kt, :], in_=qT_r[:, kt, :])
                kT_sb = ld.tile([128, DMT, sk], BF, name="kT_sb", tag="xk")
                kT_r = kT_d.rearrange("(t p) n -> p t n", p=128)
                for co, cw in _chunks(sk, 1024):
                    nc.sync.dma_start(
                        out=kT_sb[:, :, co : co + cw], in_=kT_r[:, :, co : co + cw]
                    )
                vT_sb = ld.tile([128, DMT, sk], BF, name="vT_sb", tag="xv")
                vT_r = vT_d.rearrange("(t p) n -> p t n", p=128)
                for co, cw in _chunks(sk, 512):
                    nc.sync.dma_start(
                        out=vT_sb[:, :, co : co + cw], in_=vT_r[:, :, co : co + cw]
                    )

                # Q: qhT[hd, q] = Wq.T @ q.T
                for t in range(HDT):
                    ps = psA.tile([128, 1024], F32, name=f"ps_q{t}", tag="A")
                    for kt in range(DMT):
                        nc.tensor.matmul(
                            ps[:, :sq],
                            lhsT=wq_sb[:, kt, t * 128 : (t + 1) * 128],
                            rhs=qT_sb[:, kt, :],
                            start=(kt == 0),
                            stop=(kt == DMT - 1),
                        )
                    nc.vector.tensor_scalar(
                        out=qhT[:, t, :],
                        in0=ps[:, :sq],
                        scalar1=bq_sb[:, t : t + 1],
                        scalar2=None,
                        op0=ALU.add,
                    )

                # K: khT[hd, k] = Wk.T @ k.T
                wk_sb = ld.tile([128, DMT, hd], BF, name="wk_sb", tag="w", bufs=2)
                wk_r = Wk_d.rearrange("(t p) n -> p t n", p=128)
                for kt in range(DMT):
                    nc.sync.dma_start(out=wk_sb[:, kt, :], in_=wk_r[:, kt, :])
                for co, cw in _chunks(sk, 1024):
                    for t in range(HDT):
                        ps = psA.tile([128, 1024], F32, name=f"ps_k{t}_{co}", tag="A")
                        for kt in range(DMT):
                            for no, nw in _chunks(cw, 512):
                                nc.tensor.matmul(
                                    ps[:, no : no + nw],
                                    lhsT=wk_sb[:, kt, t * 128 : (t + 1) * 128],
                                    rhs=kT_sb[:, kt, co + no : co + no + nw],
                                    start=(kt == 0),
                                    stop=(kt == DMT - 1),
                                )
                        nc.vector.tensor_scalar(
                            out=khT[:, t, co : co + cw],
                            in0=ps[:, :cw],
                            scalar1=bk_sb[:, t : t + 1],
                            scalar2=None,
                            op0=ALU.add,
                        )

                # V: vh[k_rows, hd] = (v.T)^T @ Wv  (sequence-major output)
                wv_sb = ld.tile([128, DMT, hd], BF, name="wv_sb", tag="w", bufs=2)
                wv_r = Wv_d.rearrange("(t p) n -> p t n", p=128)
                for kt in range(DMT):
                    nc.sync.dma_start(out=wv_sb[:, kt, :], in_=wv_r[:, kt, :])
                for rt in range(KT):
                    ps = psA.tile([128, 1024], F32, name=f"ps_v{rt}", tag="A")
                    for kt in range(DMT):
                        for no, nw in _chunks(hd, 512):
                            nc.tensor.matmul(
                                ps[:, no : no + nw],
                                lhsT=vT_sb[:, kt, rt * 128 : (rt + 1) * 128],
                                rhs=wv_sb[:, kt, no : no + nw],
                                start=(kt == 0),
                                stop=(kt == DMT - 1),
                            )
                    nc.vector.scalar_tensor_tensor(
                        out=vh[:, rt, :],
                        in0=ps[:, :hd],
                        scalar=1.0,
                        in1=bv_bc[:, :hd],
                        op0=ALU.bypass,
                        op1=ALU.add,
                    )

            # ================= Phase 2: attention, head pair at a time ========
            # The two heads of a pair live in partition halves [0:dk] / [dk:128]
            # of one hd-tile, so their K=64 scores matmuls row-pack and their
            # M=64 attnV matmuls col-pack into concurrent PE tile positions.
            p3w = tc.alloc_tile_pool(name="ph3w", bufs=1)
            wo_sb = p3w.tile([128, HDT, dm], BF, name="wo_sb", tag="wo")
            wo_r = Wo_d.rearrange("(t p) n -> p t n", p=128)
            for t in range(HDT):
                nc.sync.dma_start(out=wo_sb[:, t, :], in_=wo_r[:, t, :])
            with (
                tc.tile_pool(name="ph2", bufs=2) as p2,
                tc.tile_pool(name="dscr", bufs=4, space="DRAM") as dscr,
            ):
                nsc = len(_chunks(sk, 1024))
                KTG = min(8, KT)  # transposed k-tiles per PSUM staging tile
                for ht in range(h // HPT):
                    heads = [ht * HPT + j for j in range(HPT)]
                    Ps, rss, rcps, rshs = [], [], [], []
                    for j, hh in enumerate(heads):
                        Ps.append([
                            p2.tile([128, sk], BF, name=f"P{hh}_{qt}", tag="Pq",
                                    bufs=HPT * QT)
                            for qt in range(QT)
                        ])
                        rss.append(p2.tile([128, QT], F32, name=f"rs{hh}", tag=f"rs{j}"))
                        rshs.append(
                            p2.tile([128, QT, nsc], F32, name=f"rsh{hh}", tag=f"rsh{j}")
                        )
                        rcps.append(
                            p2.tile([128, QT], F32, name=f"rcp{hh}", tag=f"rcp{j}")
                        )

                    # scores [q, k] -> exp -> P (+ row sums); pair row-packed
                    for qt in range(QT):
                        for ci, (co, cw) in enumerate(_chunks(sk, 1024)):
                            pss = [
                                psA.tile(
                                    [128, 1024], F32,
                                    name=f"ps_s{hh}_{qt}_{ci}", tag="A",
                                )
                                for hh in heads
                            ]
                            for no, nw in _chunks(cw, 512):
                                for j in range(HPT):
                                    po = j * dk
                                    nc.tensor.matmul(
                                        pss[j][:, no : no + nw],
                                        lhsT=qhT[po : po + dk, ht,
                                                 qt * 128 : (qt + 1) * 128],
                                        rhs=khT[po : po + dk, ht,
                                                co + no : co + no + nw],
                                        start=True,
                                        stop=True,
                                    )
                            for j in range(HPT):
                                nc.scalar.activation(
                                    out=Ps[j][qt][:, co : co + cw],
                                    in_=pss[j][:, :cw],
                                    func=AF.Exp,
                                    scale=smax_scale,
                                    accum_out=rshs[j][:, qt, ci : ci + 1],
                                )
                        for j in range(HPT):
                            nc.vector.tensor_copy(
                                out=rss[j][:, qt : qt + 1], in_=rshs[j][:, qt, 0:1]
                            )
                            for ci in range(1, nsc):
                                nc.vector.tensor_add(
                                    rss[j][:, qt : qt + 1],
                                    rss[j][:, qt : qt + 1],
                                    rshs[j][:, qt, ci : ci + 1],
                                )
                            nc.vector.reciprocal(
                                out=rcps[j][:, qt : qt + 1],
                                in_=rss[j][:, qt : qt + 1],
                            )


                    # 1/rowsum broadcast: [dk,q] per head stacked into [128,q].
                    # Per-qt DRAM round-trips so the chain completes well before
                    # the attnV eviction needs rbc.
                    rbc = p2.tile([128, sq], F32, name=f"rbc{ht}", tag="rbc")
                    for j, hh in enumerate(heads):
                        scr = dscr.tile([sq], F32, name=f"scr{hh}", tag=f"rscr{j}")
                        nc.sync.dma_start(
                            out=scr.rearrange("(t p) -> p t", p=128), in_=rcps[j]
                        )
                        nc.sync.dma_start(
                            out=rbc[j * dk : (j + 1) * dk, :], in_=bcast_ap(scr, dk)
                        )

                    # attn = P * (1/rowsum) -> DRAM (split DVE / GpSimd)
                    for j, hh in enumerate(heads):
                        for qt in range(QT):
                            at = p2.tile(
                                [128, sk], F32, name=f"at{hh}_{qt}", tag="attn"
                            )
                            eng = (
                                nc.gpsimd
                                if (gpsimd_norm and qt % 2 == 0)
                                else nc.vector
                            )
                            eng.tensor_scalar_mul(
                                at, Ps[j][qt][:, :], rcps[j][:, qt : qt + 1]
                            )
                            nc.sync.dma_start(
                                out=attn_d[hh, qt * 128 : (qt + 1) * 128, :], in_=at
                            )

                    # P^T via PE transpose (exp reused; no 2nd softmax pass)
                    PTs = [
                        p2.tile([128, KT, sq], BF, name=f"PT{hh}", tag=f"PT{j}",
                                bufs=1)
                        for j, hh in enumerate(heads)
                    ]
                    if use_transpose:
                        # qt-outer so each P[qt] tile is fully consumed (and
                        # its slot released for the next pair) as early as
                        # possible
                        for j in range(HPT):
                            for qt in range(QT):
                                for kt0 in range(0, KT, KTG):
                                    ktn = min(KTG, KT - kt0)
                                    tps = psT.tile(
                                        [128, 1024], BF,
                                        name=f"tp{heads[j]}_{kt0}_{qt}", tag="T",
                                    )
                                    for ktp in range(ktn):
                                        nc.tensor.transpose(
                                            out=tps[:, ktp * 128 : (ktp + 1) * 128],
                                            in_=Ps[j][qt][:,
                                                          (kt0 + ktp) * 128 :
                                                          (kt0 + ktp + 1) * 128],
                                            identity=ident,
                                        )
                                    # last pair: ACT takes half the evictions
                                    # so the DVE tail doesn't delay the
                                    # output projection
                                    ev = nc.vector
                                    if ht == h // HPT - 1 and qt % 2 == 0:
                                        ev = nc.scalar
                                    if ev is nc.scalar:
                                        nc.scalar.copy(
                                            out=PTs[j][:, kt0 : kt0 + ktn,
                                                       qt * 128 : (qt + 1) * 128],
                                            in_=tps[:, : ktn * 128].rearrange(
                                                "p (a b) -> p a b", b=128
                                            ),
                                        )
                                    else:
                                        nc.vector.tensor_copy(
                                            out=PTs[j][:, kt0 : kt0 + ktn,
                                                       qt * 128 : (qt + 1) * 128],
                                            in_=tps[:, : ktn * 128].rearrange(
                                                "p (a b) -> p a b", b=128
                                            ),
                                        )
                    else:
                        for j in range(HPT):
                            po = j * dk
                            for kt in range(KT):
                                pst = psV.tile(
                                    [128, sq], F32,
                                    name=f"ps_t{heads[j]}_{kt}", tag="Vt",
                                )
                                nc.tensor.matmul(
                                    pst,
                                    lhsT=khT[po : po + dk, ht,
                                             kt * 128 : (kt + 1) * 128],
                                    rhs=qhT[po : po + dk, ht, :],
                                    start=True,
                                    stop=True,
                                )
                                nc.scalar.activation(
                                    out=PTs[j][:, kt, :], in_=pst,
                                    func=AF.Exp, scale=smax_scale,
                                )

                    # out^T[dv, q] = V^T @ P^T; the pair col-packs into one
                    # PSUM bank (partition halves), evicted with one multiply
                    av = psV.tile([128, sq], F32, name=f"av{ht}", tag="V")
                    for kt in range(KT):
                        for j, hh in enumerate(heads):
                            po = j * dk
                            nc.tensor.matmul(
                                av[po : po + dk, :],
                                lhsT=vh[:, kt, hh * dk : (hh + 1) * dk],
                                rhs=PTs[j][:, kt, :],
                                start=(kt == 0),
                                stop=(kt == KT - 1),
                                tile_position=(0, po),
                                skip_group_check=True,
                            )
                    nc.vector.tensor_tensor(
                        out=outT[:, ht, :],
                        in0=av,
                        in1=rbc,
                        op=ALU.mult,
                    )

            # ============ Phase 3: output projection + residual + LN ============
            with tc.tile_pool(name="ph3", bufs=1) as p3:
                res_r = res_d.rearrange("(t p) n -> p t n", p=128)
                res_ts = []
                for qt in range(QT):
                    rt_ = p3.tile([128, dm], F32, name=f"res{qt}", tag="res",
                                  bufs=2)
                    nc.gpsimd.dma_start(out=rt_, in_=res_r[:, qt, :])
                    res_ts.append(rt_)

                bn_fmax = min(nc.vector.BN_STATS_FMAX, dm)
                nsub = dm // bn_fmax
                for qt in range(QT):
                    ps = psA.tile([128, 1024], F32, name=f"ps_y{qt}", tag="A")
                    for no, nw in _chunks(dm, 512):
                        for t in range(HDT):
                            nc.tensor.matmul(
                                ps[:, no : no + nw],
                                lhsT=outT[:, t, qt * 128 : (qt + 1) * 128],
                                rhs=wo_sb[:, t, no : no + nw],
                                start=(t == 0),
                                stop=(t == HDT - 1),
                            )
                    xt = p3.tile([128, dm], F32, name=f"xt{qt}", tag="x", bufs=2)
                    nc.vector.scalar_tensor_tensor(
                        out=xt,
                        in0=ps[:, :dm],
                        scalar=1.0,
                        in1=bo_bc,
                        op0=ALU.bypass,
                        op1=ALU.add,
                    )
                    x2 = p3.tile([128, dm], F32, name=f"x2_{qt}", tag="x2", bufs=2)
                    nc.vector.tensor_add(x2, xt, res_ts[qt])
                    stats = p3.tile(
                        [128, nsub, nc.vector.BN_STATS_DIM], F32,
                        name=f"stats{qt}", tag="stats", bufs=2,
                    )
                    x2v = x2.rearrange("p (a b) -> p a b", b=bn_fmax)
                    for sg in range(nsub):
                        nc.vector.bn_stats(
                            out=stats[:, sg, :], in_=x2v[:, sg, :]
                        )
                    mv = p3.tile(
                        [128, nc.vector.BN_AGGR_DIM], F32,
                        name=f"mv{qt}", tag="mv", bufs=2,
                    )
                    nc.vector.bn_aggr(out=mv, in_=stats)
                    st = p3.tile([128, 4], F32, name=f"st{qt}", tag="st", bufs=2)
                    nc.scalar.activation(
                        out=st[:, 0:1], in_=mv[:, 1:2], func=AF.Sqrt, bias=eps_sb
                    )
                    nc.vector.reciprocal(out=st[:, 1:2], in_=st[:, 0:1])  # rstd
                    yt = p3.tile([128, dm], F32, name=f"yt{qt}", tag="y", bufs=2)
                    nc.vector.tensor_scalar(
                        out=yt,
                        in0=x2,
                        scalar1=mv[:, 0:1],
                        scalar2=st[:, 1:2],
                        op0=ALU.subtract,
                        op1=ALU.mult,
                    )
                    nc.gpsimd.tensor_mul(yt, yt, lns_bc)
                    nc.gpsimd.tensor_add(yt, yt, lnb_bc)
                    nc.sync.dma_start(
                        out=y_d[qt * 128 : (qt + 1) * 128, :], in_=yt
                    )
            p3w.release()

    _legalize_single_wait(nc)
    return nc


_NC_CACHE = {}
LAST_RESULTS = None


def _get_nc():
    if "full" not in _NC_CACHE:
        _NC_CACHE["full"] = build_program()
    return _NC_CACHE["full"]


def kernel(q, k, v, Wq, bq, Wk, bk, Wv, bv, Wo, bo, ln_scale, ln_bias, **kw):
    global LAST_RESULTS
    q = np.asarray(q, np.float32)
    k = np.asarray(k, np.float32)
    v = np.asarray(v, np.float32)
    shared = {
        "Wq": np.ascontiguousarray(np.asarray(Wq, np.float32).astype(BF16)),
        "Wk": np.ascontiguousarray(np.asarray(Wk, np.float32).astype(BF16)),
        "Wv": np.ascontiguousarray(np.asarray(Wv, np.float32).astype(BF16)),
        "Wo": np.ascontiguousarray(np.asarray(Wo, np.float32).astype(BF16)),
        "bq": np.ascontiguousarray(np.asarray(bq, np.float32)),
        "bk": np.ascontiguousarray(np.asarray(bk, np.float32)),
        "bv": np.ascontiguousarray(np.asarray(bv, np.float32)),
        "bo": np.ascontiguousarray(np.asarray(bo, np.float32)),
        "ln_scale": np.ascontiguousarray(np.asarray(ln_scale, np.float32)),
        "ln_bias": np.ascontiguousarray(np.asarray(ln_bias, np.float32)),
    }
    in_maps = []
    nchunk = N_CORES // B
    for c in range(N_CORES):
        b = c // nchunk
        qlo = (c % nchunk) * Q_SHARD
        qs = q[b, qlo : qlo + Q_SHARD, :]
        in_maps.append(
            dict(
                shared,
                qT=np.ascontiguousarray(qs.T.astype(BF16)),
                kT=np.ascontiguousarray(k[b].T.astype(BF16)),
                vT=np.ascontiguousarray(v[b].T.astype(BF16)),
                resid=np.ascontiguousarray(qs),
            )
        )

    nc = _get_nc()
    res = run_bass_kernel_spmd(nc, in_maps, core_ids=list(range(N_CORES)))
    LAST_RESULTS = res

    y = np.empty((B, SEQ, D_MODEL), np.float32)
    attn = np.empty((N_HEAD * B, SEQ, SEQ), np.float32)
    heads = np.arange(N_HEAD) * B
    for c, om in enumerate(res.results):
        b = c // nchunk
        qlo = (c % nchunk) * Q_SHARD
        y[b, qlo : qlo + Q_SHARD] = om["y"]
        attn[heads + b, qlo : qlo + Q_SHARD, :] = om["attn"]
    return y, attn
